# revision 5
# baseline (speedup 1.0000x reference)
"""PSNR-conv kernel for TRN2 (8 NeuronCores, SPMD).

Problem: per 16x16 window of a 4x2048x2048 image, alpha-blend with a 16x16
RGBA kernel and compute PSNR vs the kernel, averaged over channels.
Mathematically per channel c:
    mse_c = sum_ij w'_ij * (x_c[i,j] - k_c[i,j])^2,   w' = ((1-a)^2)/256
          = conv(x_c^2, w') - conv(x_c, 2 k_c w') + sum(k_c^2 w')
    out   = A - B * sum_c ln(mse_c),  A = 20 log10(255), B = 10/(4 ln 10)

Implementation: each depthwise 16x16 VALID conv is computed as 16 banded-
Toeplitz matmuls on the PE array (contraction over 128 input rows, lhsT =
Toeplitz of one kernel column, rhs = image tile shifted by dj in the free
dim), accumulating both conv planes (x^2 and x) into one PSUM tile so the
PSUM holds mse_c - skk_c directly. ScalarE Ln(+skk bias) + VectorE combine
produce the output tile.

Sharding: 2x4 grid (2 row strips x 4 col strips). Every core runs an
identical instruction stream (SPMD, same NEFF): 9 row blocks of 113 output
rows x 509 output cols. Strips overlap by a few rows/cols (recomputed) so
all cores are uniform. Inputs are fp32r (e8m11) for 1 cycle/row matmuls.
"""

import sys

if "/opt/trn_rl_repo" not in sys.path:
    sys.path.insert(0, "/opt/trn_rl_repo")

import numpy as np

PIXEL_MAX = 255.0
C, Hk, Wk = 4, 16, 16
H = W = 2048
HO = WO = H - Hk + 1          # 2033
MB = 113                      # output rows per block (128 - 15)
KP = 128                      # contraction size (input rows per block)
NRB = 9                       # row blocks per core; 9*113 = 1017 rows
OUT_ROWS = NRB * MB           # 1017
NCOL = 510                    # output cols per core (fp32r mm: must be even)
IN_COLS = NCOL + Hk - 1       # 525
IN_ROWS = OUT_ROWS + Hk - 1   # 1032
ROW_STARTS = [0, HO - OUT_ROWS]                    # [0, 1016]
COL_STARTS = [0, 507, 1015, WO - NCOL]             # [0, 507, 1015, 1523]
N_CORES = 8

A_CONST = 20.0 * np.log10(PIXEL_MAX)
B_CONST = 10.0 / (4.0 * np.log(10.0))


def _toeplitz(col):
    """[128, 113] banded Toeplitz T[k, m] = col[k - m] for 0 <= k-m < 16."""
    t = np.zeros((KP, MB), np.float32)
    for d in range(Hk):
        idx = np.arange(MB)
        t[idx + d, idx] = col[d]
    return t


def _build_nc(skk):
    import concourse.bacc as bacc
    import concourse.mybir as mybir
    from concourse.tile import TileContext

    f32 = mybir.dt.float32
    f32r = mybir.dt.float32r
    Ln = mybir.ActivationFunctionType.Ln
    mult = mybir.AluOpType.mult
    add = mybir.AluOpType.add

    nc = bacc.Bacc("TRN2", target_bir_lowering=False, debug=False)

    # Register const APs for the per-channel skk activation biases (the
    # ACT bias-as-float path looks immediates up in const_aps).
    for v in skk:
        v = float(v)
        t = nc.alloc_sbuf_tensor(f"const-float32-{v}", [128, 1], f32)
        nc.gpsimd.memset(t.ap(), v)
        nc.const_aps.aps[(f32, v)] = t.ap()
    nc.all_engine_barrier()

    xs = nc.dram_tensor("xs", [C, IN_ROWS, IN_COLS], f32, kind="ExternalInput")
    tw = nc.dram_tensor("tw", [Hk, KP, MB], f32, kind="ExternalInput")
    tkw = nc.dram_tensor("tkw", [C, Hk, KP, MB], f32, kind="ExternalInput")
    out = nc.dram_tensor("out", [OUT_ROWS, NCOL], f32, kind="ExternalOutput")

    with TileContext(nc) as tc:
        with (
            tc.tile_pool(name="wpool", bufs=1) as wpool,
            tc.tile_pool(name="xpool", bufs=2) as xpool,
            tc.tile_pool(name="lnpool", bufs=2) as lnpool,
            tc.tile_pool(name="opool", bufs=2) as opool,
            tc.tile_pool(name="pspool", bufs=2, space="PSUM") as pspool,
        ):
            # one-time: weights -> SBUF, round to fp32r
            tw_st = wpool.tile([KP, Hk * MB], f32)
            tkw_st = wpool.tile([KP, C * Hk * MB], f32)
            nc.sync.dma_start(
                tw_st[:].rearrange("k (d m) -> k d m", d=Hk),
                tw[:].rearrange("d k m -> k d m"),
            )
            nc.sync.dma_start(
                tkw_st[:].rearrange("k (c d m) -> k c d m", c=C, d=Hk),
                tkw[:].rearrange("c d k m -> k c d m"),
            )
            twr = wpool.tile([KP, Hk * MB], f32r)
            tkwr = wpool.tile([KP, C * Hk * MB], f32r)
            nc.vector.tensor_copy(twr[:], tw_st[:])
            nc.vector.tensor_copy(tkwr[:], tkw_st[:])

            for rb in range(NRB):
                row0 = MB * rb
                xt = xpool.tile([KP, C * IN_COLS], f32, tag="xt")
                nc.sync.dma_start(
                    xt[:].rearrange("r (c w) -> r c w", c=C),
                    xs[:, row0:row0 + KP, :].rearrange("c r w -> r c w"),
                )
                xxr = xpool.tile([KP, C * IN_COLS], f32r, tag="xxr")
                xr = xpool.tile([KP, C * IN_COLS], f32r, tag="xr")
                nc.vector.tensor_mul(xxr[:], xt[:], xt[:])
                nc.vector.tensor_copy(xr[:], xt[:])

                lns = []
                for c in range(C):
                    ps = pspool.tile([MB, NCOL], f32, tag=f"ps{c}", name=f"ps{c}")
                    for dj in range(Hk):
                        nc.tensor.matmul(
                            ps[:],
                            twr[:, dj * MB:(dj + 1) * MB],
                            xxr[:, c * IN_COLS + dj: c * IN_COLS + dj + NCOL],
                            start=(dj == 0), stop=False,
                        )
                    for dj in range(Hk):
                        nc.tensor.matmul(
                            ps[:],
                            tkwr[:, (c * Hk + dj) * MB:(c * Hk + dj + 1) * MB],
                            xr[:, c * IN_COLS + dj: c * IN_COLS + dj + NCOL],
                            start=False, stop=(dj == Hk - 1),
                        )
                    lnc = lnpool.tile([MB, NCOL], f32, tag=f"ln{c}", name=f"ln{c}")
                    nc.scalar.activation(
                        lnc[:], ps[:], Ln, bias=float(skk[c]), scale=1.0
                    )
                    lns.append(lnc)

                s01 = opool.tile([MB, NCOL], f32, tag="s01")
                s23 = opool.tile([MB, NCOL], f32, tag="s23")
                nc.vector.tensor_add(s01[:], lns[0][:], lns[1][:])
                nc.vector.tensor_add(s23[:], lns[2][:], lns[3][:])
                t = opool.tile([MB, NCOL], f32, tag="t")
                nc.vector.tensor_add(t[:], s01[:], s23[:])
                ob = opool.tile([MB, NCOL], f32, tag="ob")
                nc.vector.tensor_scalar(
                    ob[:], t[:], -B_CONST, A_CONST, mult, add
                )
                nc.sync.dma_start(out[row0:row0 + MB, :], ob[:])

    nc.compile()
    return nc


def kernel(x, kernel):
    from concourse.bass_utils import run_bass_kernel_spmd

    x = np.asarray(x)
    kernel = np.asarray(kernel)
    k = kernel[0].astype(np.float64)                    # (4, 16, 16)
    alpha = k[3] / PIXEL_MAX
    wp = ((1.0 - alpha) ** 2) / (Hk * Wk)               # w' = (1-a)^2 / 256

    tw_np = np.zeros((Hk, KP, MB), np.float32)
    tkw_np = np.zeros((C, Hk, KP, MB), np.float32)
    for dj in range(Hk):
        tw_np[dj] = _toeplitz(wp[:, dj].astype(np.float32))
        for c in range(C):
            tkw_np[c, dj] = _toeplitz((-2.0 * k[c, :, dj] * wp[:, dj]).astype(np.float32))
    skk = (k * k * wp).sum(axis=(-2, -1))               # (4,)

    nc = _build_nc(skk)

    x0 = np.ascontiguousarray(x[0], np.float32)         # (4, 2048, 2048)
    in_maps = []
    for r in range(2):
        for cc in range(4):
            r0, c0 = ROW_STARTS[r], COL_STARTS[cc]
            in_maps.append({
                "xs": np.ascontiguousarray(
                    x0[:, r0:r0 + IN_ROWS, c0:c0 + IN_COLS]
                ),
                "tw": tw_np,
                "tkw": tkw_np,
            })

    res = run_bass_kernel_spmd(nc, in_maps, core_ids=list(range(N_CORES)))

    full = np.empty((HO, WO), np.float32)
    for r in range(2):
        for cc in range(4):
            core = r * 4 + cc
            r0, c0 = ROW_STARTS[r], COL_STARTS[cc]
            full[r0:r0 + OUT_ROWS, c0:c0 + NCOL] = res.results[core]["out"]
    return full
